# revision 55
# baseline (speedup 1.0000x reference)
"""Distributed kNN OOD-score kernel for 8 Trainium2 NeuronCores.

Problem: for each of 4*32*32 query vectors (D=768), find the 3 nearest
database vectors (N=20000, squared-L2), average the 3 distances, and
bilinearly upsample the resulting [4,32,32] map to [4,1,512,512].

Sharding: queries are data-parallel. Each core owns half of one batch
image (16 of 32 query rows = 512 queries); the database is replicated
and streamed through SBUF in fp8-e4m3. The bilinear upsample needs one
halo row from the pair core, but its contribution is additive and only
touches the 8 output rows at the block boundary - so each core also
emits an [8,512] partial strip (its boundary row interpolated onto the
pair's output rows) and the HOST adds the strips while unsharding. No
cross-core communication happens on device at all.

Scoring: t[q,n] = q.x - ||x||^2/2 via fp8 DoubleRow matmuls (K=256 per
pass, 3 passes, 2x column rate = 4x bf16 throughput). The contraction
carries 766 real dims plus two "slot" rows holding -||x||^2/2 in
split-fp8 (query-side scales 2 and 1), so the norm term rides along in
band; the two dropped embedding dims only lose their cross terms
(zero-mean noise ~2.8 in d^2, irrelevant at rel-err 2e-2).

Selection: top-3 per query = max over the (negated-distance-monotone)
scores. PSUM holds 4 rotating [128,2,512] f32 tiles (two hardware banks
each; matmul outputs must not straddle the 2KB banks). Each 1000-col
tile is drained by one of two statically assigned paths:
  - DVE direct: windowed tensor_reduce(max, win 20) PSUM -> bf16 strip.
  - Act evac: ScalarE copies the tile to bf16 SBUF; DVE runs a win-8
    tensor_max tree in the 2-byte 2x mode (two wide levels per group,
    final level at qt end).
A final DVE max8 per query tile yields the top-3 scores; ScalarE turns
them into distances (fused sqrt((q^2-2t)/9)) and DVE reduce_sum
averages. GPSIMD cannot touch PSUM nor run TensorTensor on TRN2, so it
only runs memsets.

Upsample: 16x bilinear = two small bf16 matmuls (1 cycle/row) with
host-built interpolation matrices. The halo partial strip is produced
early (it only needs qt0, the boundary block) and the per-qt ood
columns are assembled incrementally, so the post-selection tail is just
ood asm -> p1 -> p2 -> output DMA.
"""

import sys

if "/opt/trn_rl_repo" not in sys.path:
    sys.path.insert(0, "/opt/trn_rl_repo")

import numpy as np
import ml_dtypes

import concourse.bass as bass
import concourse.bacc as bacc
import concourse.mybir as mybir
import concourse.tile as tile
from concourse import bass_utils

# Problem shape (hardcoded per contract).
B, D, H, W = 4, 768, 32, 32
N = 20000
K_NN = 3
OUT_H = OUT_W = 512
N_CORES = 8

QPC = 512            # queries scored per core (16 rows)
N_QT = QPC // 128    # 4 query tiles
PASSES = 3           # fp8 DoubleRow K=256 contraction passes
DREAL = 766          # real embedding dims carried (dims 766,767 -> slots)
GCOLS = 4000         # db columns per DMA group
N_G = N // GCOLS     # 5
UCOLS = 1000         # db columns per PSUM tile (drain unit)
UPG = GCOLS // UCOLS  # 4 units per group
N_U = N // UCOLS     # 20 units per query tile
OROWS = 256          # output rows per core
NCOL = 16            # ood columns entering the upsample (own rows only)

# drain-path assignment: units (of 20 per qt) drained by DVE directly;
# the rest are evacuated by ScalarE and tree-maxed on DVE.
DVE_UNITS = (0, 3, 7, 10, 13, 17)
DWIN = 100                       # DVE direct window -> 10 strip cols/unit
DSTRIP = UCOLS // DWIN           # 50
ASTRIP = UCOLS // 8              # 125 strip cols per Act unit (win-8 tree)

F32 = mybir.dt.float32
F32R = mybir.dt.float32r
BF16 = mybir.dt.bfloat16
FP8 = mybir.dt.float8e4
E4M3 = ml_dtypes.float8_e4m3
AX = mybir.AxisListType
AF = mybir.ActivationFunctionType
DR = mybir.MatmulPerfMode.DoubleRow
MAX = mybir.AluOpType.max

# local tile -> 4-row block of this core's half (block i = rows 4i..4i+3).
# Tile 0 is the block the PAIR core needs as its halo row: for the top
# half (rows 0-15) that's block 3 (row 15), for the bottom half (rows
# 16-31) block 0 (row 16).
TILE_BLOCKS = ([3, 0, 1, 2], [0, 1, 2, 3])

# --- static unit bookkeeping ------------------------------------------------
# Units 0..15 land in groups 0-3 ("A" phase), 16..19 in group 4 ("B").
DVE_A = [u for u in range(16) if u in DVE_UNITS]          # 5 units
ACT_A = [u for u in range(16) if u not in DVE_UNITS]      # 11 units
DVE_B = [u for u in range(16, N_U) if u in DVE_UNITS]     # 1 unit
ACT_B = [u for u in range(16, N_U) if u not in DVE_UNITS]  # 3 units
N_ACT = len(ACT_A) + len(ACT_B)                            # 14

# strip layout (same for every qt):
#   [dveA 250][actA 1375][dveB 50][actB 375]            W=2050


def _strip_layout(qt):
    off = {}
    p = 0
    for u in DVE_A:
        off[u] = p
        p += DSTRIP
    for i, u in enumerate(ACT_A):
        off[u] = p + i * ASTRIP
    p += len(ACT_A) * ASTRIP
    t8 = None
    if qt == N_QT - 1:
        # qt3 folds its groups-0-3 strip region into a top-8 slot during
        # the steady state, so its (wall-clock-critical) tail max8 only
        # scans the group-4 region
        t8 = p
        p += 8
    for u in DVE_B:
        off[u] = p
        p += DSTRIP
    actb = p
    for i, u in enumerate(ACT_B):
        off[u] = p + i * ASTRIP
    p += len(ACT_B) * ASTRIP
    return off, t8, actb, p


_TMP_OFF = {u: i * 250 for i, u in enumerate(ACT_A + ACT_B)}


def _build_program():
    nc = bacc.Bacc(
        "TRN2", target_bir_lowering=False, debug=False, num_devices=N_CORES
    )
    dbx = nc.dram_tensor("dbx", [D, N], FP8, kind="ExternalInput").ap()
    qx = nc.dram_tensor(
        "qx", [128, PASSES, N_QT, 2, 128], FP8, kind="ExternalInput"
    ).ap()
    q2 = nc.dram_tensor("q2", [128, N_QT], F32, kind="ExternalInput").ap()
    art = nc.dram_tensor("art", [NCOL, OROWS], BF16, kind="ExternalInput").ap()
    ac4 = nc.dram_tensor("ac4", [128, OUT_W], BF16, kind="ExternalInput").ap()
    m4 = nc.dram_tensor("m4", [128, 4], BF16, kind="ExternalInput").ap()
    art_hp = nc.dram_tensor("art_hp", [4, 8], BF16, kind="ExternalInput").ap()
    out = nc.dram_tensor("out", [OROWS, OUT_W], F32, kind="ExternalOutput").ap()
    halo = nc.dram_tensor("halo", [8, OUT_W], F32, kind="ExternalOutput").ap()

    layouts = [_strip_layout(qt) for qt in range(N_QT)]

    with tile.TileContext(nc) as tc:
        with (
            tc.tile_pool(name="static", bufs=1) as sp,
            tc.tile_pool(name="db", bufs=6) as dbp,
            tc.tile_pool(name="evac", bufs=6) as evp,
            tc.tile_pool(name="tmp5", bufs=3) as t5p,
            tc.tile_pool(name="small", bufs=4) as smp,
            tc.tile_pool(name="psum", bufs=4, space="PSUM") as pp,
            tc.tile_pool(name="dram", bufs=1, space="DRAM") as dp,
        ):
            q_sb = sp.tile([128, PASSES, N_QT, 2, 128], FP8)
            nc.sync.dma_start(q_sb[:], qx[:])
            # warmup junk + masks memset FIRST so the PE p-state warmup
            # isn't gated behind Pool's SWDGE input-DMA issues
            junkq = sp.tile([128, 2, 128], FP8)
            nc.gpsimd.memset(junkq[:], 0.0)
            junk = sp.tile([128, 2, 512], FP8)
            nc.gpsimd.memset(junk[:], 0.0)
            warm1 = sp.tile([128, 1], F32)
            nc.gpsimd.memset(warm1[:], 1.0)
            lmask = [
                sp.tile([128, 8], BF16, name=f"lmask{i}") for i in range(2)
            ]
            nc.gpsimd.memset(lmask[0][:], 0.0)
            nc.gpsimd.memset(lmask[1][:], 0.0)
            # the small epilogue inputs load via the idle SWDGE queue so
            # they don't delay the db stream on SP
            q2_sb = sp.tile([128, N_QT], F32)
            nc.gpsimd.dma_start(q2_sb[:], q2[:])
            art_lo = sp.tile([8, OROWS], BF16)
            nc.gpsimd.dma_start(art_lo[:], art[0:8, :])
            art_hi = sp.tile([8, OROWS], BF16)
            nc.gpsimd.dma_start(art_hi[:], art[8:NCOL, :])
            ac4_sb = sp.tile([128, OUT_W], BF16)
            nc.gpsimd.dma_start(ac4_sb[:], ac4[:])
            m4_sb = sp.tile([128, 4], BF16)
            nc.gpsimd.dma_start(m4_sb[:], m4[:])
            art_hp_sb = sp.tile([4, 8], BF16)
            nc.gpsimd.dma_start(art_hp_sb[:], art_hp[:])

            # PE p-state warmup on junk data (no input dependencies), and
            # a dummy Sqrt to pull the activation-table load off the
            # critical path of qt0's distance epilogue.
            warm_o = sp.tile([128, 1], F32)
            nc.scalar.activation(warm_o[:], warm1[:], AF.Sqrt)
            nc.scalar.activation(warm_o[:], warm1[:], AF.Copy)
            for wu in range(28):
                wt = pp.tile([128, 2, 512], F32, tag="ps", name="warm")
                nc.tensor.matmul(
                    wt[:, 0, :], junkq[:], junk[:], start=True, stop=True,
                    perf_mode=DR,
                )

            strips = [
                sp.tile([128, layouts[qt][3]], BF16, name=f"strip{qt}")
                for qt in range(N_QT)
            ]
            tmp250 = [
                sp.tile([128, N_ACT * 250], BF16, name=f"t250_{qt}")
                for qt in range(N_QT)
            ]
            oods = [
                sp.tile([128, 1], F32, name=f"ood{qt}") for qt in range(N_QT)
            ]
            # stream the db by column group; group 0 is fetched in two
            # half-width rounds so its first units can start sooner
            db_tiles = {}
            for g in range(N_G):
                for j in range(PASSES):
                    t = dbp.tile([128, 2, GCOLS], FP8, tag="db", name=f"db{g}_{j}")
                    db_tiles[(g, j)] = t
            for g, half in [(0, 0), (0, 1)] + [(g, None) for g in range(1, N_G)]:
                for j in range(PASSES):
                    t = db_tiles[(g, j)]
                    src_ap = dbx[
                        256 * j : 256 * (j + 1),
                        g * GCOLS : (g + 1) * GCOLS,
                    ].rearrange("(i p) c -> p i c", i=2)
                    if half is None:
                        nc.sync.dma_start(t[:], src_ap)
                    else:
                        lo, hi = half * GCOLS // 2, (half + 1) * GCOLS // 2
                        nc.sync.dma_start(t[:, :, lo:hi], src_ap[:, :, lo:hi])

            def qt_end(qt):
                """Emit qt's final selection + distance epilogue."""
                off, t8, actb, wid = layouts[qt]
                tm = tmp250[qt][:].rearrange("p (a c) -> p a c", c=250)
                st = strips[qt]
                na, nb = len(ACT_A), len(ACT_B)
                if t8 is None:
                    nc.vector.tensor_max(
                        st[:, off[ACT_A[0]] : off[ACT_A[0]] + na * ASTRIP]
                        .rearrange("p (a c) -> p a c", c=ASTRIP),
                        tm[:, 0:na, 0:125],
                        tm[:, 0:na, 125:250],
                    )
                    m8_in = st[:]
                else:
                    # A-region already folded into the t8 slot mid-stream
                    m8_in = st[:, t8:wid]
                nc.vector.tensor_max(
                    st[:, actb : actb + nb * ASTRIP].rearrange(
                        "p (a c) -> p a c", c=ASTRIP
                    ),
                    tm[:, na : na + nb, 0:125],
                    tm[:, na : na + nb, 125:250],
                )
                f8 = smp.tile([128, 8], BF16, tag="f8", name="f8")
                nc.vector.max(f8[:], m8_in)
                # dist_j/3 = sqrt((q2 - 2 t_j)/9); host passes q2/9
                d3 = smp.tile([128, K_NN], F32, tag="d3", name="d3")
                nc.scalar.activation(
                    d3[:],
                    f8[:, 0:K_NN],
                    AF.Sqrt,
                    bias=q2_sb[:, qt : qt + 1],
                    scale=-2.0 / 9.0,
                )
                nc.vector.reduce_sum(oods[qt][:], d3[:], axis=AX.X)
                # scatter this qt's ood values into the masked stationary
                # operand of the upsample transpose-matmul: L[p, 4(qt%2)+r]
                # = ood[p] * (p//32 == r)
                half_t = lmask[qt // 2]
                nc.vector.tensor_scalar_mul(
                    half_t[:, 4 * (qt % 2) : 4 * (qt % 2) + 4],
                    m4_sb[:],
                    oods[qt][:, 0:1],
                )

            for g in range(N_G):
                for j in range(PASSES):
                    t = dbp.tile([128, 2, GCOLS], FP8, tag="db", name=f"db{g}_{j}")
                    db_tiles[(g, j)] = t
            for g, half in [(0, 0), (0, 1)] + [(g, None) for g in range(1, N_G)]:
                for j in range(PASSES):
                    t = db_tiles[(g, j)]
                    src_ap = dbx[
                        256 * j : 256 * (j + 1),
                        g * GCOLS : (g + 1) * GCOLS,
                    ].rearrange("(i p) c -> p i c", i=2)
                    if half is None:
                        nc.sync.dma_start(t[:], src_ap)
                    else:
                        lo, hi = half * GCOLS // 2, (half + 1) * GCOLS // 2
                        nc.sync.dma_start(t[:, :, lo:hi], src_ap[:, :, lo:hi])

            ood_hT = sp.tile([W, NCOL], F32)

            def qt_end(qt):
                """Emit qt's final selection + distance epilogue."""
                off, t8, actb, wid = layouts[qt]
                tm = tmp250[qt][:].rearrange("p (a c) -> p a c", c=250)
                st = strips[qt]
                na, nb = len(ACT_A), len(ACT_B)
                if qt == 0:
                    # only the B-phase level 3 + tail max8 (A was folded
                    # into the t8 slot at group 3)
                    nc.vector.tensor_max(
                        st[:, actb : actb + nb * ASTRIP].rearrange(
                            "p (a c) -> p a c", c=ASTRIP
                        ),
                        tm[:, na : na + nb, 0:125],
                        tm[:, na : na + nb, 125:250],
                    )
                    m8_in = st[:, t8:wid]
                else:
                    nc.vector.tensor_max(
                        st[:, off[ACT_A[0]] : off[ACT_A[0]] + na * ASTRIP]
                        .rearrange("p (a c) -> p a c", c=ASTRIP),
                        tm[:, 0:na, 0:125],
                        tm[:, 0:na, 125:250],
                    )
                    nc.vector.tensor_max(
                        st[:, actb : actb + nb * ASTRIP].rearrange(
                            "p (a c) -> p a c", c=ASTRIP
                        ),
                        tm[:, na : na + nb, 0:125],
                        tm[:, na : na + nb, 125:250],
                    )
                    m8_in = st[:]
                f8 = smp.tile([128, 8], BF16, tag="f8", name="f8")
                nc.vector.max(f8[:], m8_in)
                # dist_j/3 = sqrt((q2 - 2 t_j)/9); host passes q2/9
                d3 = smp.tile([128, K_NN], F32, tag="d3", name="d3")
                nc.scalar.activation(
                    d3[:],
                    f8[:, 0:K_NN],
                    AF.Sqrt,
                    bias=q2_sb[:, qt : qt + 1],
                    scale=-2.0 / 9.0,
                )
                nc.vector.reduce_sum(oods[qt][:], d3[:], axis=AX.X)
                if qt == 0:
                    # boundary block: gather it across the pair ASAP so
                    # the collective hides under the remaining tails
                    nc.sync.dma_start(cc_in[:], oods[0][:])
                    nc.gpsimd.collective_compute(
                        "AllGather",
                        mybir.AluOpType.bypass,
                        replica_groups=[[0, 1], [2, 3], [4, 5], [6, 7]],
                        ins=[cc_in.opt()],
                        outs=[cc_out.opt()],
                    )
                nc.sync.dma_start(
                    scratch.rearrange("(q p) -> p q", p=128)[:, qt : qt + 1],
                    oods[qt][:],
                )

            pending_trees = []
            for g in range(N_G):
                for qt in range(N_QT):
                    off, t8, actb, wid = layouts[qt]
                    acts = [
                        h for h in range(UPG)
                        if (g * UPG + h) not in DVE_UNITS
                    ]
                    ev = None
                    if acts:
                        ev = evp.tile([128, 4, UCOLS], BF16, tag="ev", name="ev")
                    for h in range(UPG):
                        u = g * UPG + h
                        ps = pp.tile([128, 2, 512], F32, tag="ps", name="ps")
                        for j in range(PASSES):
                            for bk in range(2):
                                c0 = h * UCOLS + bk * 500
                                nc.tensor.matmul(
                                    ps[:, bk, 0:500],
                                    q_sb[:, j, qt],
                                    db_tiles[(g, j)][:, :, c0 : c0 + 500],
                                    start=(j == 0),
                                    stop=(j == PASSES - 1),
                                    perf_mode=DR,
                                )
                        if u in DVE_UNITS:
                            nc.vector.tensor_reduce(
                                strips[qt][:, off[u] : off[u] + DSTRIP],
                                ps[:, :, 0:500].rearrange(
                                    "p b (w k) -> p b w k", k=DWIN
                                ),
                                axis=AX.X,
                                op=MAX,
                            )
                        else:
                            nc.scalar.activation(
                                ev[:, acts.index(h), :], ps[:, :, 0:500], AF.Copy
                            )
                    if acts:
                        # defer this group's tree by one (g,qt) slot so
                        # DVE's in-order queue doesn't head-of-line block
                        # on Act's evacs while later PSUM drains are ready
                        def make_tree(qt=qt, acts=acts, ev=ev, g=g):
                            def emit():
                                nr = len(acts)
                                t5 = t5p.tile(
                                    [128, 4, 500], BF16, tag="t5", name="t5"
                                )
                                nc.vector.tensor_max(
                                    t5[:, 0:nr, :],
                                    ev[:, 0:nr, 0:500],
                                    ev[:, 0:nr, 500:1000],
                                )
                                a0 = _TMP_OFF[g * UPG + acts[0]]
                                nc.vector.tensor_max(
                                    tmp250[qt][:, a0 : a0 + nr * 250].rearrange(
                                        "p (a c) -> p a c", c=250
                                    ),
                                    t5[:, 0:nr, 0:250],
                                    t5[:, 0:nr, 250:500],
                                )
                            return emit
                        pending_trees.append(make_tree())
                    if g == 3 and qt == N_QT - 1:
                        def fold_qt3():
                            offq, t8q, _actb, _w = layouts[N_QT - 1]
                            stq = strips[N_QT - 1]
                            tmq = tmp250[N_QT - 1][:].rearrange(
                                "p (a c) -> p a c", c=250
                            )
                            na = len(ACT_A)
                            nc.vector.tensor_max(
                                stq[:, offq[ACT_A[0]] : offq[ACT_A[0]] + na * ASTRIP]
                                .rearrange("p (a c) -> p a c", c=ASTRIP),
                                tmq[:, 0:na, 0:125],
                                tmq[:, 0:na, 125:250],
                            )
                            nc.vector.max(
                                stq[:, t8q : t8q + 8], stq[:, 0:t8q]
                            )
                        pending_trees.append(fold_qt3)
                    while len(pending_trees) > 1:
                        pending_trees.pop(0)()
                    if g == N_G - 1:
                        while pending_trees:
                            pending_trees.pop(0)()
                        qt_end(qt)

            # Upsample, split into a qt0/qt1 half (ready well before the
            # last query tile finishes) and a qt2/qt3 half; only the last
            # half's short chain sits on the critical path.
            # p1[j, ow] = sum_p L[p, j] * ac4[p, ow]
            #           = sum_c ood[32*(j%4)+c] * A_c[c, ow]
            p1a = pp.tile([8, OUT_W], F32, tag="ps", name="p1a")
            p1a_sb = sp.tile([8, OUT_W], BF16)
            nc.tensor.matmul(
                p1a[:], lmask[0][:], ac4_sb[:], start=True, stop=True
            )
            nc.scalar.activation(p1a_sb[:], p1a[:], AF.Copy)
            # halo partial: this core's boundary row (tile 0) interpolated
            # onto the pair's 8 edge output rows; the host sums it into
            # the pair's output while unsharding.
            ph = pp.tile([8, OUT_W], F32, tag="ps", name="ph")
            nc.tensor.matmul(
                ph[:], art_hp_sb[:], p1a_sb[0:4, :], start=True, stop=True
            )
            ph_sb = sp.tile([8, OUT_W], F32)
            nc.scalar.activation(ph_sb[:], ph[:], AF.Copy)
            nc.gpsimd.dma_start(halo[:], ph_sb[:])
            p2 = [
                pp.tile([128, OUT_W], F32, tag="ps", name=f"p2_{m}")
                for m in range(2)
            ]
            for m in range(2):
                nc.tensor.matmul(
                    p2[m][:],
                    art_lo[:, m * 128 : (m + 1) * 128],
                    p1a_sb[:],
                    start=True,
                    stop=False,
                )

            p1b = pp.tile([8, OUT_W], F32, tag="ps", name="p1b")
            p1b_sb = sp.tile([8, OUT_W], BF16)
            nc.tensor.matmul(
                p1b[:], lmask[1][:], ac4_sb[:], start=True, stop=True
            )
            nc.scalar.activation(p1b_sb[:], p1b[:], AF.Copy)
            o_sb = sp.tile([128, 2, OUT_W], F32)
            for m in range(2):
                nc.tensor.matmul(
                    p2[m][:],
                    art_hi[:, m * 128 : (m + 1) * 128],
                    p1b_sb[:],
                    start=False,
                    stop=True,
                )
                nc.scalar.activation(o_sb[:, m, :], p2[m][:], AF.Copy)
                eng = nc.gpsimd if m == 0 else nc.sync
                eng.dma_start(
                    out[m * 128 : (m + 1) * 128, :], o_sb[:, m, :]
                )

    nc.compile()
    return nc


def _bilinear_matrix(out_size: int, in_size: int) -> np.ndarray:
    """Half-pixel (align_corners=False) bilinear interpolation matrix
    [out_size, in_size]; edge-clamped, equivalent to jax.image.resize
    'bilinear' for integer upsampling."""
    A = np.zeros((out_size, in_size), dtype=np.float64)
    scale = in_size / out_size
    for i in range(out_size):
        s = (i + 0.5) * scale - 0.5
        j0 = int(np.floor(s))
        w = s - j0
        A[i, min(max(j0, 0), in_size - 1)] += 1.0 - w
        A[i, min(max(j0 + 1, 0), in_size - 1)] += w
    return A.astype(np.float32)


_NC_CACHE = None


def _get_nc():
    global _NC_CACHE
    if _NC_CACHE is None:
        _NC_CACHE = _build_program()
    return _NC_CACHE


def make_in_maps(embeddings: np.ndarray, database: np.ndarray):
    embeddings = np.asarray(embeddings, dtype=np.float32)
    database = np.asarray(database, dtype=np.float32)

    # db fp8 layout: rows 0..765 = dims, rows 766/767 = -||x||^2/2 in
    # split fp8 with query-side scales (2, 1)
    r = -0.5 * np.einsum("nd,nd->n", database, database)
    slotA = (r / 2.0).astype(E4M3)
    slotB = (r - 2.0 * slotA.astype(np.float32)).astype(E4M3)
    dbx = np.empty((D, N), dtype=E4M3)
    dbx[0:DREAL] = np.ascontiguousarray(database.T[0:DREAL]).astype(E4M3)
    dbx[DREAL] = slotA
    dbx[DREAL + 1] = slotB

    q_all = embeddings.transpose(0, 2, 3, 1).reshape(B, H * W, D)
    Ac = _bilinear_matrix(OUT_W, W)                      # [512, 32]
    Ar = _bilinear_matrix(OUT_H, H)                      # [512, 32]

    in_maps = []
    for c in range(N_CORES):
        b, half = divmod(c, 2)
        blocks = TILE_BLOCKS[half]
        own_rows = [16 * half + 4 * blk + r_ for blk in blocks for r_ in range(4)]

        # queries in local-tile order
        q = np.concatenate(
            [
                q_all[b, (16 * half + 4 * blk) * W : (16 * half + 4 * blk + 4) * W]
                for blk in blocks
            ]
        )                                                # [512, 768]
        Qx = np.empty((D, QPC), dtype=E4M3)
        Qx[0:DREAL] = np.ascontiguousarray(q.T[0:DREAL]).astype(E4M3)
        Qx[DREAL] = 2.0
        Qx[DREAL + 1] = 1.0
        # device layout qx[p, j, qt, i, m] = Qx[256j + 128i + p, 128qt + m]
        qxb = np.ascontiguousarray(
            Qx.reshape(PASSES, 2, 128, N_QT, 128).transpose(2, 0, 3, 1, 4)
        )
        q2 = np.einsum("qd,qd->q", q, q) / 9.0
        q2 = np.ascontiguousarray(q2.reshape(N_QT, 128).T.astype(np.float32))

        # interpolation rows matching ood_hT's column order (own rows only)
        Arh = Ar[half * OROWS : (half + 1) * OROWS]      # [256, 32]
        art = np.zeros((NCOL, OROWS), dtype=np.float32)
        for j, row in enumerate(own_rows):
            art[j] = Arh[:, row]
        # halo partial: weight of this core's boundary row (local tile-0
        # row bnd_j, global row bnd_g) on the pair core's 8 edge output
        # rows. The pair's edge rows are the first 8 of its 256-row range
        # for the top half, the last 8 for the bottom half.
        art_hp = np.zeros((4, 8), dtype=np.float32)
        if half == 0:
            bnd_j, bnd_g = 3, 15            # tile0 = block 3 = rows 12..15
            pair_rows = Ar[OROWS : OROWS + 8]
        else:
            bnd_j, bnd_g = 0, 16            # tile0 = block 0 = rows 16..19
            pair_rows = Ar[OROWS - 8 : OROWS]
        art_hp[bnd_j] = pair_rows[:, bnd_g]
        m4v = np.zeros((128, 4), dtype=np.float32)
        m4v[np.arange(128), np.arange(128) // 32] = 1.0
        in_maps.append(
            {
                "dbx": dbx,
                "qx": qxb,
                "q2": q2,
                "art": art.astype(ml_dtypes.bfloat16),
                "ac4": np.ascontiguousarray(
                    np.tile(Ac.T, (4, 1))
                ).astype(ml_dtypes.bfloat16),
                "m4": m4v.astype(ml_dtypes.bfloat16),
                "art_hp": art_hp.astype(ml_dtypes.bfloat16),
            }
        )
    return in_maps


def run_device(in_maps, **kwargs):
    nc = _get_nc()
    return bass_utils.run_bass_kernel_spmd(
        nc, in_maps, core_ids=list(range(N_CORES)), **kwargs
    )


def kernel(embeddings, database, k, out_h, out_w):
    assert int(k) == K_NN and int(out_h) == OUT_H and int(out_w) == OUT_W
    in_maps = make_in_maps(np.asarray(embeddings), np.asarray(database))
    res = run_device(in_maps)
    out = np.empty((B, 1, OUT_H, OUT_W), dtype=np.float32)
    for c in range(N_CORES):
        b, half = divmod(c, 2)
        out[b, 0, half * OROWS : (half + 1) * OROWS] = res.results[c]["out"]
    # add the cross-boundary halo partials (device computes them; the
    # host-side unshard sums them into the pair core's edge rows)
    for b in range(B):
        even, odd = 2 * b, 2 * b + 1
        out[b, 0, OROWS : OROWS + 8] += res.results[even]["halo"]
        out[b, 0, OROWS - 8 : OROWS] += res.results[odd]["halo"]
    return out
